# revision 2
# baseline (speedup 1.0000x reference)
"""Trainium2 Bass kernel for nn_DecoderRNN (LSTM decoder + vocab projection).

Strategy (8 NeuronCores, cost-model-driven rewrite):
  - Vocab-shard the output projection 8-way; replicate the LSTM recurrence.
  - Gate-major recurrence: gates.T [gate_dims, batch] so every matmul runs
    with a full M=128 stationary tile; moving dim N=64 (batch).  fp8-e4m3
    DoubleRow matmuls contract 2 K-planes per instruction at 0.5 cyc/row.
  - Scales: emb*8 (fp8), W_ih.T*16 (fp8), h unscaled (fp8), W_hh.T*128
    (fp8); PSUM holds 128*preact; activations apply scale=1/128.
    Timestep 0 (image features) runs in bf16 at scale 1.
  - Gate tiles ordered (f,i,g,o) so ACT runs 4 merged ops per step:
    SIG_fi [128,512], TANH_g [128,256], SIG_o [128,256], TANH_c [128,256].
  - Elementwise on DVE as bf16 tensor_tensor (2x mode); h written twice:
    fp8 (recurrence) + bf16 (h_seqT for the fc projection).
  - fc projection (bf16, N-chunks 512/512/226 in one 3-bank PSUM tile)
    paced into the scan; drained to bf16 staging split ACT/DVE; DMA out.
  - Embeddings gathered on-device from a host-prescaled fp8 table,
    transposed via the PE (fp8 identity), paced ahead of the scan.
"""

import numpy as np
import ml_dtypes

import concourse.bacc as bacc
import concourse.mybir as mybir
import concourse.tile as tile
from concourse.bass import IndirectOffsetOnAxis
from concourse.bass_utils import run_bass_kernel_spmd
from concourse.masks import make_identity

B, T, E, H, V = 64, 32, 512, 512, 10000
G4 = 4 * H            # 2048 gate dims (f,i,g,o ordered)
NTOK = B * T          # 2048 tokens
NCORES = 8
VL = V // NCORES      # 1250 vocab per core
KH = H // 128         # 4 K-chunks
NCH = NTOK // 128     # 16 gather chunks
OOB = 1 << 30         # gather sentinel for t=0 rows

SX = 8.0              # emb fp8 scale
SWX = 16.0            # W_ih fp8 scale
SWH = 128.0           # W_hh fp8 scale (h fp8 unscaled)
PS = SX * SWX         # PSUM preact scale for t>=1

F32 = mybir.dt.float32
BF16 = mybir.dt.bfloat16
FP8 = mybir.dt.float8e4
I32 = mybir.dt.int32
AFT = mybir.ActivationFunctionType
ALU = mybir.AluOpType
DR = mybir.MatmulPerfMode.DoubleRow

FC_CHUNKS = [(0, 512), (512, 512), (1024, VL - 1024)]


def build_nc(with_gate_bias: bool, with_fc_bias: bool):
    nc = bacc.Bacc("TRN2", target_bir_lowering=False, debug=False,
                   num_devices=NCORES)

    emb_d = nc.dram_tensor("emb8", [V, E], BF16, kind="ExternalInput")
    idx_d = nc.dram_tensor("idx", [128, NCH], I32, kind="ExternalInput")
    ft_d = nc.dram_tensor("featT", [KH, 128, B], F32, kind="ExternalInput")
    wco_d = nc.dram_tensor("wco", [2 * KH, 128, G4], FP8, kind="ExternalInput")
    wx0_d = nc.dram_tensor("wx0", [KH, 128, G4], BF16, kind="ExternalInput")
    fct_d = nc.dram_tensor("fct", [KH, 128, VL], BF16, kind="ExternalInput")
    bg_d = nc.dram_tensor("bg", [1, G4], F32, kind="ExternalInput")
    fcb_d = nc.dram_tensor("fcb", [1, VL], F32, kind="ExternalInput")
    out_d = nc.dram_tensor("out", [NTOK, VL], BF16, kind="ExternalOutput")
    import os
    dbg = {}
    if os.environ.get("K2_DEBUG"):
        dbg["xsT01"] = nc.dram_tensor("dbg_xsT01", [128, KH * 256], FP8, kind="ExternalOutput")
        dbg["h0"] = nc.dram_tensor("dbg_h0", [128, KH * B], FP8, kind="ExternalOutput")
        dbg["sig1"] = nc.dram_tensor("dbg_sig1", [128, 512], BF16, kind="ExternalOutput")
        dbg["gx1"] = nc.dram_tensor("dbg_gx1", [128, 1024], BF16, kind="ExternalOutput")
        dbg["gh1"] = nc.dram_tensor("dbg_gh1", [128, 1024], BF16, kind="ExternalOutput")
        dbg["g1"] = nc.dram_tensor("dbg_g1", [128, 1024], BF16, kind="ExternalOutput")

    with tile.TileContext(nc) as tc:
        build_body(nc, tc, emb_d, idx_d, ft_d, wco_d, wx0_d, fct_d, bg_d,
                   fcb_d, out_d, with_gate_bias, with_fc_bias, dbg)
    nc.compile()
    return nc


def build_body(nc, tc, emb_d, idx_d, ft_d, wco_d, wx0_d, fct_d, bg_d, fcb_d,
               out_d, with_gate_bias, with_fc_bias, dbg={}):
    from contextlib import ExitStack
    ctx = ExitStack()
    with ctx:
        const = ctx.enter_context(tc.tile_pool(name="const", bufs=1))
        gst = ctx.enter_context(tc.tile_pool(name="gst", bufs=3))
        ew = ctx.enter_context(tc.tile_pool(name="ew", bufs=2))
        ost = ctx.enter_context(tc.tile_pool(name="ost", bufs=2))
        gatesp = ctx.enter_context(tc.tile_pool(name="gatesp", bufs=2, space="PSUM"))
        projp = ctx.enter_context(tc.tile_pool(name="projp", bufs=1, space="PSUM"))
        tpsum = ctx.enter_context(tc.tile_pool(name="tpsum", bufs=1, space="PSUM"))

        # ---- constants / weights ----
        identb = const.tile([128, 128], BF16)
        make_identity(nc, identb[:])
        wrm = const.tile([128, 512], BF16)
        nc.gpsimd.memset(wrm[:], 0.25)

        idx_sb = const.tile([128, NCH], I32)
        nc.sync.dma_start(idx_sb[:], idx_d.ap())
        ftst = const.tile([128, KH * B], F32)
        for k in range(KH):
            nc.sync.dma_start(ftst[:, k * B:(k + 1) * B], ft_d[k])
        wx0 = const.tile([128, KH, G4], BF16)
        for k in range(KH):
            nc.sync.dma_start(wx0[:, k, :], wx0_d[k])
        w8 = const.tile([128, 2 * KH, G4], FP8)
        fct = const.tile([128, KH, VL], BF16)

        if with_gate_bias:
            bgrow = const.tile([1, G4], F32)
            nc.sync.dma_start(bgrow[:], bg_d.ap())
            bg128 = const.tile([1, G4], BF16)
            nc.vector.tensor_scalar_mul(bg128[:], bgrow[:], PS)
            bg1 = const.tile([1, G4], BF16)
            nc.vector.tensor_copy(bg1[:], bgrow[:])
            ones64 = const.tile([1, B], BF16)
            nc.gpsimd.memset(ones64[:], 1.0)
        if with_fc_bias:
            fcbrow = const.tile([1, VL], F32)
            nc.sync.dma_start(fcbrow[:], fcb_d.ap())
            fcb_sb = const.tile([1, VL], BF16)
            nc.vector.tensor_copy(fcb_sb[:], fcbrow[:])
            ones128 = const.tile([1, 128], BF16)
            nc.gpsimd.memset(ones128[:], 1.0)

        wps = gatesp.tile([128, 1024], F32, name="gates", tag="gates")
        for w_i in range(10):
            nc.tensor.matmul(wps[:, 0:512], lhsT=identb[:], rhs=wrm[:],
                             start=(w_i == 0), stop=(w_i == 9),
                             skip_group_check=True)

        xsT = const.tile([128, KH, NTOK], FP8)     # emb.T fp8, plane k
        xsT0 = const.tile([128, KH * B], BF16)     # features.T bf16
        h_seqT = const.tile([128, KH, NTOK], BF16)  # h.T bf16 for fc

        # ---- helpers ----
        drain_flip = [0]

        def gather_chunk(m):
            gs = gst.tile([128, E], BF16, name="gs", tag="gs")
            if m == 0:
                nc.gpsimd.memset(gs[:], 0.0)
            nc.gpsimd.indirect_dma_start(
                out=gs[:], out_offset=None, in_=emb_d.ap(),
                in_offset=IndirectOffsetOnAxis(ap=idx_sb[:, m:m + 1], axis=0),
                bounds_check=V - 1, oob_is_err=False,
            )
            tp = tpsum.tile([128, E], BF16, name="tp", tag="tp")
            for k in range(KH):
                nc.tensor.transpose(tp[:, k * 128:(k + 1) * 128],
                                    gs[:, k * 128:(k + 1) * 128], identb[:])
            dst = xsT[:, :, m * 128:(m + 1) * 128]
            src = tp[:].rearrange("p (k n) -> p k n", k=KH)
            if drain_flip[0] % 2 == 0:
                nc.scalar.copy(dst, src)
            else:
                nc.vector.tensor_copy(dst, src)
            drain_flip[0] += 1

        def xmm(t, gates):
            # x-part for step t (fp8 DR), opens each tile's PSUM group
            for g in range(16):
                for p in range(2):
                    nc.tensor.matmul(
                        gates[:, g * B:(g + 1) * B],
                        lhsT=w8[:, 2 * p:2 * p + 2, g * 128:(g + 1) * 128],
                        rhs=xsT[:, 2 * p:2 * p + 2, t * B:(t + 1) * B],
                        start=(p == 0 and g in (0, 8)), stop=False,
                        perf_mode=DR, skip_group_check=True,
                    )
            if with_gate_bias:
                for g in range(16):
                    nc.tensor.matmul(
                        gates[:, g * B:(g + 1) * B],
                        lhsT=bg128[:, g * 128:(g + 1) * 128],
                        rhs=ones64[:],
                        start=False, stop=False, skip_group_check=True,
                    )

        def hmm(t, gates, hT):
            # h-part for step t (fp8 DR), closes each tile's group.
            # q-major so q=0 runs while the second half of hT is produced;
            # f,i tiles (g 0..7) close first so SIG_fi starts earliest.
            for q in range(2):
                for g in range(16):
                    nc.tensor.matmul(
                        gates[:, g * B:(g + 1) * B],
                        lhsT=w8[:, KH + 2 * q:KH + 2 * q + 2,
                                g * 128:(g + 1) * 128],
                        rhs=hT[:, 2 * q:2 * q + 2, :],
                        start=False, stop=(q == 1),
                        perf_mode=DR, skip_group_check=True,
                    )

        state = {"c": None, "hT": None}

        def elementwise(t, gates):
            scale = 1.0 if t == 0 else 1.0 / PS
            sig_fi = ew.tile([128, 512], BF16, name="sig_fi", tag="sfi")
            nc.scalar.activation(sig_fi[:], gates[:, 0:512], AFT.Sigmoid,
                                 scale=scale)
            tg = ew.tile([128, 256], BF16, name="tg", tag="tg")
            nc.scalar.activation(tg[:], gates[:, 512:768], AFT.Tanh,
                                 scale=scale)
            so = ew.tile([128, 256], BF16, name="so", tag="so")
            nc.scalar.activation(so[:], gates[:, 768:1024], AFT.Sigmoid,
                                 scale=scale)
            c_new = ew.tile([128, 256], BF16, name="c", tag="c")
            if t == 0:
                nc.vector.tensor_tensor(c_new[:], sig_fi[:, 256:512], tg[:],
                                        op=ALU.mult)
            else:
                fc_ = ew.tile([128, 256], BF16, name="fc_", tag="fc_")
                nc.vector.tensor_tensor(fc_[:], sig_fi[:, 0:256],
                                        state["c"][:], op=ALU.mult)
                u = ew.tile([128, 256], BF16, name="u", tag="u")
                nc.vector.tensor_tensor(u[:], sig_fi[:, 256:512], tg[:],
                                        op=ALU.mult)
                nc.vector.tensor_tensor(c_new[:], fc_[:], u[:], op=ALU.add)
            tc_t = ew.tile([128, 256], BF16, name="tc_t", tag="tc_t")
            nc.scalar.activation(tc_t[:], c_new[:], AFT.Tanh)
            hT = ew.tile([128, KH, B], FP8, name="hT", tag="hT")
            for hh in range(2):
                nc.vector.tensor_tensor(
                    hT[:, 2 * hh:2 * hh + 2, :],
                    so[:, 128 * hh:128 * (hh + 1)]
                    .rearrange("p (k n) -> p k n", k=2),
                    tc_t[:, 128 * hh:128 * (hh + 1)]
                    .rearrange("p (k n) -> p k n", k=2),
                    op=ALU.mult)
            nc.vector.tensor_tensor(
                h_seqT[:, :, t * B:(t + 1) * B],
                so[:].rearrange("p (k n) -> p k n", k=KH),
                tc_t[:].rearrange("p (k n) -> p k n", k=KH),
                op=ALU.mult)
            state["c"] = c_new
            state["hT"] = hT
            state["sig1"] = sig_fi

        fc_state = {}

        def fc_mms(m, chunks):
            pj = fc_state.get(m)
            if pj is None:
                pj = fc_state[m] = projp.tile([128, 1536], F32, name="pj",
                                              tag="pj")
            for ci in chunks:
                n0, nsz = FC_CHUNKS[ci]
                for k in range(KH):
                    nc.tensor.matmul(
                        pj[:, n0:n0 + nsz],
                        lhsT=h_seqT[:, k, m * 128:(m + 1) * 128],
                        rhs=fct[:, k, n0:n0 + nsz],
                        start=(k == 0),
                        stop=(k == KH - 1) and not with_fc_bias,
                        skip_group_check=True,
                    )
                if with_fc_bias:
                    nc.tensor.matmul(
                        pj[:, n0:n0 + nsz], lhsT=ones128[:],
                        rhs=fcb_sb[:, n0:n0 + nsz],
                        start=False, stop=True, skip_group_check=True,
                    )

        def fc_finish(m):
            pj = fc_state.pop(m)
            half = 640
            st_a = ost.tile([128, half], BF16, name="st_a", tag="st_a")
            st_b = ost.tile([128, VL - half], BF16, name="st_b", tag="st_b")
            nc.scalar.copy(st_a[:], pj[:, 0:half])
            nc.vector.tensor_copy(st_b[:], pj[:, half:VL])
            nc.sync.dma_start(out_d[m * 128:(m + 1) * 128, 0:half], st_a[:])
            nc.sync.dma_start(out_d[m * 128:(m + 1) * 128, half:VL], st_b[:])

        # ---- prologue ----
        gather_chunk(0)
        gather_chunk(1)
        for k in range(2 * KH):
            nc.sync.dma_start(w8[:, k, :], wco_d[k])
        gather_chunk(2)
        gather_chunk(3)
        for k in range(KH):
            nc.sync.dma_start(fct[:, k, :], fct_d[k])
        nc.scalar.copy(xsT0[:], ftst[:])  # f32 -> bf16 cast

        gates0 = gatesp.tile([128, 1024], F32, name="gates", tag="gates")
        for k in range(KH):
            for g in range(16):
                nc.tensor.matmul(
                    gates0[:, g * B:(g + 1) * B],
                    lhsT=wx0[:, k, g * 128:(g + 1) * 128],
                    rhs=xsT0[:, k * B:(k + 1) * B],
                    start=(k == 0 and g in (0, 8)),
                    stop=(k == KH - 1) and not with_gate_bias,
                    skip_group_check=True,
                )
        if with_gate_bias:
            for g in range(16):
                nc.tensor.matmul(
                    gates0[:, g * B:(g + 1) * B],
                    lhsT=bg1[:, g * 128:(g + 1) * 128], rhs=ones64[:],
                    start=False, stop=True, skip_group_check=True,
                )
        elementwise(0, gates0)

        gates_next = gatesp.tile([128, 1024], F32, name="gates", tag="gates")
        xmm(1, gates_next)

        # ---- scan ----
        if dbg:
            for k in range(KH):
                nc.sync.dma_start(dbg["h0"].ap()[:, k * B:(k + 1) * B],
                                  state["hT"][:, k, :])

        for t in range(1, T):
            gates = gates_next
            hmm(t, gates, state["hT"])
            if dbg and t == 1:
                gtmp = const.tile([128, 1024], BF16)
                nc.vector.tensor_copy(gtmp[:], gates[:])
                nc.sync.dma_start(dbg["g1"].ap(), gtmp[:])
                # recompute x-part and h-part separately into fresh psum
                for nm, base in (("gx1", 0), ("gh1", KH)):
                    gd = gatesp.tile([128, 1024], F32, name="gates", tag="gates")
                    for g2 in range(16):
                        for p in range(2):
                            rhs = (xsT[:, 2 * p:2 * p + 2, 1 * B:2 * B] if base == 0
                                   else state["hT"][:, 2 * p:2 * p + 2, :])
                            nc.tensor.matmul(
                                gd[:, g2 * B:(g2 + 1) * B],
                                lhsT=w8[:, base + 2 * p:base + 2 * p + 2,
                                        g2 * 128:(g2 + 1) * 128],
                                rhs=rhs, start=(p == 0), stop=(p == 1),
                                perf_mode=DR, skip_group_check=True)
                    gt2 = const.tile([128, 1024], BF16, name=f"gt_{nm}")
                    nc.vector.tensor_copy(gt2[:], gd[:])
                    nc.sync.dma_start(dbg[nm].ap(), gt2[:])
            if t % 2 == 0:
                fc_mms(t // 2 - 1, (0, 1))
            elif t >= 3:
                fc_mms((t - 3) // 2, (2,))
            if t < T - 1:
                gates_next = gatesp.tile([128, 1024], F32, name="gates",
                                         tag="gates")
                xmm(t + 1, gates_next)
            if t % 2 == 0 and 4 <= t // 2 + 3 < NCH:
                gather_chunk(t // 2 + 3)
            elementwise(t, gates)
            if t % 2 == 1 and t >= 3:
                fc_finish((t - 3) // 2)
            if dbg and t == 1:
                nc.sync.dma_start(dbg["sig1"].ap(), state["sig1"][:])

        if dbg:
            for k in range(KH):
                nc.sync.dma_start(dbg["xsT01"].ap()[:, k * 256:(k + 1) * 256],
                                  xsT[:, k, 0:256])
        # keep the PE p-state ramped through the final chain window so the
        # last projection prices at full clock
        warm = gatesp.tile([128, 1024], F32, name="gates", tag="gates")
        for w_i in range(14):
            nc.tensor.matmul(
                warm[:, 0:512], lhsT=wx0[:, w_i % KH, 0:128],
                rhs=fct[:, w_i % KH, 0:512],
                start=(w_i == 0), stop=(w_i == 13), skip_group_check=True,
            )
        fc_mms(NCH - 1, (0, 1, 2))
        fc_finish(NCH - 1)


_CACHE = {}


def _get_nc(with_gate_bias, with_fc_bias):
    key = (with_gate_bias, with_fc_bias)
    if key not in _CACHE:
        _CACHE[key] = build_nc(with_gate_bias, with_fc_bias)
    return _CACHE[key]


LAST_RESULTS = None

# gate reorder: pytorch rows (i,f,g,o) -> kernel order (f,i,g,o)
_PERM = np.concatenate([np.arange(H, 2 * H), np.arange(0, H),
                        np.arange(2 * H, 3 * H), np.arange(3 * H, 4 * H)])


def kernel(features, captions, embed_W, W_ih, W_hh, b_ih, b_hh, fc_W, fc_b,
           _trace=False):
    global LAST_RESULTS
    features = np.asarray(features, dtype=np.float32)
    captions = np.asarray(captions)
    embed_W = np.asarray(embed_W, dtype=np.float32)
    W_ih = np.asarray(W_ih, dtype=np.float32)
    W_hh = np.asarray(W_hh, dtype=np.float32)
    b_ih = np.asarray(b_ih, dtype=np.float32)
    b_hh = np.asarray(b_hh, dtype=np.float32)
    fc_W = np.asarray(fc_W, dtype=np.float32)
    fc_b = np.asarray(fc_b, dtype=np.float32)

    with_gate_bias = bool(np.any(b_ih) or np.any(b_hh))
    with_fc_bias = bool(np.any(fc_b))
    nc = _get_nc(with_gate_bias, with_fc_bias)

    FP8N = ml_dtypes.float8_e4m3fn
    BFN = ml_dtypes.bfloat16

    emb8 = (embed_W * SX).astype(BFN)

    # token-major indices, column m = tokens [128m, 128m+128); t=0 rows OOB
    tok = np.arange(NTOK)
    tt_, bb = tok // B, tok % B
    idx = np.where(tt_ == 0, OOB,
                   captions.astype(np.int64)[bb, tt_].astype(np.int64)
                   ).astype(np.int32)
    idx = np.ascontiguousarray(idx.reshape(NCH, 128).T)

    featT = np.ascontiguousarray(features.T.reshape(KH, 128, B))

    wxT = np.ascontiguousarray(W_ih.T[:, _PERM])          # [E, G4] reordered
    whT = np.ascontiguousarray(W_hh.T[:, _PERM])          # [H, G4]
    wco = np.concatenate([
        (wxT * SWX).astype(FP8N).reshape(KH, 128, G4),
        (whT * SWH).astype(FP8N).reshape(KH, 128, G4)], axis=0)
    wx0 = wxT.astype(BFN).reshape(KH, 128, G4)

    bg = (b_ih + b_hh)[_PERM].reshape(1, G4)

    fcT_full = np.ascontiguousarray(fc_W.T)               # [H, V]

    in_maps = []
    for c in range(NCORES):
        fct_c = np.ascontiguousarray(
            fcT_full[:, c * VL:(c + 1) * VL]).astype(BFN).reshape(KH, 128, VL)
        in_maps.append({
            "emb8": emb8,
            "idx": idx,
            "featT": featT,
            "wco": wco,
            "wx0": wx0,
            "fct": fct_c,
            "bg": bg,
            "fcb": fc_b[c * VL:(c + 1) * VL].reshape(1, VL),
        })

    try:
        res = run_bass_kernel_spmd(nc, in_maps, list(range(NCORES)),
                                   trace=_trace)
    except ModuleNotFoundError:
        res = run_bass_kernel_spmd(nc, in_maps, list(range(NCORES)))
    LAST_RESULTS = res

    outs = [
        np.asarray(res.results[c]["out"]).astype(np.float32)
        .reshape(T, B, VL).transpose(1, 0, 2)
        for c in range(NCORES)
    ]
    return np.ascontiguousarray(np.concatenate(outs, axis=2),
                                dtype=np.float32)


# revision 4
# speedup vs baseline: 1.0052x; 1.0052x over previous
"""Trainium2 Bass kernel for nn_DecoderRNN (LSTM decoder + vocab projection).

Strategy (8 NeuronCores, cost-model-driven rewrite):
  - Vocab-shard the output projection 8-way; replicate the LSTM recurrence.
  - Gate-major recurrence: gates.T [gate_dims, batch] so every matmul runs
    with a full M=128 stationary tile; moving dim N=64 (batch).  fp8-e4m3
    DoubleRow matmuls contract 2 K-planes per instruction at 0.5 cyc/row.
  - Scales: emb*8 (fp8), W_ih.T*16 (fp8), h unscaled (fp8), W_hh.T*128
    (fp8); PSUM holds 128*preact; activations apply scale=1/128.
    Timestep 0 (image features) runs in bf16 at scale 1.
  - Gate tiles ordered (f,i,g,o) so ACT runs 4 merged ops per step:
    SIG_fi [128,512], TANH_g [128,256], SIG_o [128,256], TANH_c [128,256].
  - Elementwise on DVE as bf16 tensor_tensor (2x mode); h written twice:
    fp8 (recurrence) + bf16 (h_seqT for the fc projection).
  - fc projection (bf16, N-chunks 512/512/226 in one 3-bank PSUM tile)
    paced into the scan (chunks 0-1 on even steps, chunk 2 + drain + DMA
    on odd steps); drained to two bf16 staging tiles split ACT/DVE.
  - Embeddings gathered on-device from a host-prescaled bf16 table,
    transposed via the PE, cast to fp8 in the drain, paced ahead of the
    scan.  One PSUM-bank start per matmul group (tiles 0 and 8) -- the
    hardware zeroes a whole 2KB bank per start_tensor_calc.
  - Post-TANH_g elementwise runs as two pipelined [128,128] half-chains
    so the h-part matmuls of the next step start on the first half while
    the second half finishes.  PE keep-warm matmuls before the tail
    projection hold the p-state at full clock.
"""

import numpy as np
import ml_dtypes

import concourse.bacc as bacc
import concourse.mybir as mybir
import concourse.tile as tile
from concourse.bass import IndirectOffsetOnAxis
from concourse.bass_utils import run_bass_kernel_spmd
from concourse.masks import make_identity

B, T, E, H, V = 64, 32, 512, 512, 10000
G4 = 4 * H            # 2048 gate dims (f,i,g,o ordered)
NTOK = B * T          # 2048 tokens
NCORES = 8
VL = V // NCORES      # 1250 vocab per core
KH = H // 128         # 4 K-chunks
NCH = NTOK // 128     # 16 gather chunks
OOB = 1 << 30         # gather sentinel for t=0 rows

SX = 8.0              # emb fp8 scale
SWX = 16.0            # W_ih fp8 scale
SWH = 128.0           # W_hh fp8 scale (h fp8 unscaled)
PS = SX * SWX         # PSUM preact scale for t>=1

F32 = mybir.dt.float32
BF16 = mybir.dt.bfloat16
FP8 = mybir.dt.float8e4
I32 = mybir.dt.int32
AFT = mybir.ActivationFunctionType
ALU = mybir.AluOpType
DR = mybir.MatmulPerfMode.DoubleRow

FC_CHUNKS = [(0, 512), (512, 512), (1024, VL - 1024)]


def build_nc(with_gate_bias: bool, with_fc_bias: bool):
    nc = bacc.Bacc("TRN2", target_bir_lowering=False, debug=False,
                   num_devices=NCORES)

    emb_d = nc.dram_tensor("emb8", [V, E], BF16, kind="ExternalInput")
    idx_d = nc.dram_tensor("idx", [128, NCH], I32, kind="ExternalInput")
    ft_d = nc.dram_tensor("featT", [KH, 128, B], F32, kind="ExternalInput")
    wco_d = nc.dram_tensor("wco", [2 * KH, 128, G4], FP8, kind="ExternalInput")
    wx0_d = nc.dram_tensor("wx0", [KH, 128, G4], BF16, kind="ExternalInput")
    fct_d = nc.dram_tensor("fct", [KH, 128, VL], BF16, kind="ExternalInput")
    bg_d = nc.dram_tensor("bg", [1, G4], F32, kind="ExternalInput")
    fcb_d = nc.dram_tensor("fcb", [1, VL], F32, kind="ExternalInput")
    out_d = nc.dram_tensor("out", [NTOK, VL], BF16, kind="ExternalOutput")
    import os
    dbg = {}
    if os.environ.get("K2_DEBUG"):
        dbg["xsT01"] = nc.dram_tensor("dbg_xsT01", [128, KH * 256], FP8, kind="ExternalOutput")
        dbg["h0"] = nc.dram_tensor("dbg_h0", [128, KH * B], FP8, kind="ExternalOutput")
        dbg["sig1"] = nc.dram_tensor("dbg_sig1", [128, 512], BF16, kind="ExternalOutput")
        dbg["gx1"] = nc.dram_tensor("dbg_gx1", [128, 1024], BF16, kind="ExternalOutput")
        dbg["gh1"] = nc.dram_tensor("dbg_gh1", [128, 1024], BF16, kind="ExternalOutput")
        dbg["g1"] = nc.dram_tensor("dbg_g1", [128, 1024], BF16, kind="ExternalOutput")

    with tile.TileContext(nc) as tc:
        build_body(nc, tc, emb_d, idx_d, ft_d, wco_d, wx0_d, fct_d, bg_d,
                   fcb_d, out_d, with_gate_bias, with_fc_bias, dbg)
    nc.compile()
    return nc


def build_body(nc, tc, emb_d, idx_d, ft_d, wco_d, wx0_d, fct_d, bg_d, fcb_d,
               out_d, with_gate_bias, with_fc_bias, dbg={}):
    from contextlib import ExitStack
    ctx = ExitStack()
    with ctx:
        const = ctx.enter_context(tc.tile_pool(name="const", bufs=1))
        gst = ctx.enter_context(tc.tile_pool(name="gst", bufs=3))
        ew = ctx.enter_context(tc.tile_pool(name="ew", bufs=2))
        ost = ctx.enter_context(tc.tile_pool(name="ost", bufs=2))
        gatesp = ctx.enter_context(tc.tile_pool(name="gatesp", bufs=2, space="PSUM"))
        projp = ctx.enter_context(tc.tile_pool(name="projp", bufs=1, space="PSUM"))
        tpsum = ctx.enter_context(tc.tile_pool(name="tpsum", bufs=1, space="PSUM"))

        # ---- constants / weights ----
        identb = const.tile([128, 128], BF16)
        make_identity(nc, identb[:])
        wrm = const.tile([128, 512], BF16)
        nc.gpsimd.memset(wrm[:], 0.25)

        idx_sb = const.tile([128, NCH], I32)
        nc.sync.dma_start(idx_sb[:], idx_d.ap())
        ftst = const.tile([128, KH * B], F32)
        for k in range(KH):
            nc.sync.dma_start(ftst[:, k * B:(k + 1) * B], ft_d[k])
        wx0 = const.tile([128, KH, G4], BF16)
        for k in range(KH):
            nc.sync.dma_start(wx0[:, k, :], wx0_d[k])
        w8 = const.tile([128, 2 * KH, G4], FP8)
        fct = const.tile([128, KH, VL], BF16)

        if with_gate_bias:
            bgrow = const.tile([1, G4], F32)
            nc.sync.dma_start(bgrow[:], bg_d.ap())
            bg128 = const.tile([1, G4], BF16)
            nc.vector.tensor_scalar_mul(bg128[:], bgrow[:], PS)
            bg1 = const.tile([1, G4], BF16)
            nc.vector.tensor_copy(bg1[:], bgrow[:])
            ones64 = const.tile([1, B], BF16)
            nc.gpsimd.memset(ones64[:], 1.0)
        if with_fc_bias:
            fcbrow = const.tile([1, VL], F32)
            nc.sync.dma_start(fcbrow[:], fcb_d.ap())
            fcb_sb = const.tile([1, VL], BF16)
            nc.vector.tensor_copy(fcb_sb[:], fcbrow[:])
            ones128 = const.tile([1, 128], BF16)
            nc.gpsimd.memset(ones128[:], 1.0)

        wps = gatesp.tile([128, 1024], F32, name="gates", tag="gates")
        for w_i in range(10):
            nc.tensor.matmul(wps[:, 0:512], lhsT=identb[:], rhs=wrm[:],
                             start=(w_i == 0), stop=(w_i == 9),
                             skip_group_check=True)

        xsT = const.tile([128, KH, NTOK], FP8)     # emb.T fp8, plane k
        xsT0 = const.tile([128, KH * B], BF16)     # features.T bf16
        h_seqT = const.tile([128, KH, NTOK], BF16)  # h.T bf16 for fc

        # ---- helpers ----
        drain_flip = [0]

        def gather_chunk(m):
            gs = gst.tile([128, E], BF16, name="gs", tag="gs")
            if m == 0:
                nc.gpsimd.memset(gs[:], 0.0)
            nc.gpsimd.indirect_dma_start(
                out=gs[:], out_offset=None, in_=emb_d.ap(),
                in_offset=IndirectOffsetOnAxis(ap=idx_sb[:, m:m + 1], axis=0),
                bounds_check=V - 1, oob_is_err=False,
            )
            tp = tpsum.tile([128, E], BF16, name="tp", tag="tp")
            for k in range(KH):
                nc.tensor.transpose(tp[:, k * 128:(k + 1) * 128],
                                    gs[:, k * 128:(k + 1) * 128], identb[:])
            dst = xsT[:, :, m * 128:(m + 1) * 128]
            src = tp[:].rearrange("p (k n) -> p k n", k=KH)
            if drain_flip[0] % 2 == 0:
                nc.scalar.copy(dst, src)
            else:
                nc.vector.tensor_copy(dst, src)
            drain_flip[0] += 1

        def xmm(t, gates):
            # x-part for step t (fp8 DR), opens each tile's PSUM group
            for g in range(16):
                for p in range(2):
                    nc.tensor.matmul(
                        gates[:, g * B:(g + 1) * B],
                        lhsT=w8[:, 2 * p:2 * p + 2, g * 128:(g + 1) * 128],
                        rhs=xsT[:, 2 * p:2 * p + 2, t * B:(t + 1) * B],
                        start=(p == 0 and g in (0, 8)), stop=False,
                        perf_mode=DR, skip_group_check=True,
                    )
            if with_gate_bias:
                for g in range(16):
                    nc.tensor.matmul(
                        gates[:, g * B:(g + 1) * B],
                        lhsT=bg128[:, g * 128:(g + 1) * 128],
                        rhs=ones64[:],
                        start=False, stop=False, skip_group_check=True,
                    )

        def hmm(t, gates, hT):
            # h-part for step t (fp8 DR), closes each tile's group.
            # q-major so q=0 runs while the second half of hT is produced;
            # f,i tiles (g 0..7) close first so SIG_fi starts earliest.
            for q in range(2):
                for g in range(16):
                    nc.tensor.matmul(
                        gates[:, g * B:(g + 1) * B],
                        lhsT=w8[:, KH + 2 * q:KH + 2 * q + 2,
                                g * 128:(g + 1) * 128],
                        rhs=hT[:, 2 * q:2 * q + 2, :],
                        start=False, stop=(q == 1),
                        perf_mode=DR, skip_group_check=True,
                    )

        state = {"c": None, "hT": None}

        def elementwise(t, gates):
            scale = 1.0 if t == 0 else 1.0 / PS
            sig_fi = ew.tile([128, 512], BF16, name="sig_fi", tag="sfi")
            nc.scalar.activation(sig_fi[:], gates[:, 0:512], AFT.Sigmoid,
                                 scale=scale)
            tg = ew.tile([128, 256], BF16, name="tg", tag="tg")
            nc.scalar.activation(tg[:], gates[:, 512:768], AFT.Tanh,
                                 scale=scale)
            so = ew.tile([128, 256], BF16, name="so", tag="so")
            nc.scalar.activation(so[:], gates[:, 768:1024], AFT.Sigmoid,
                                 scale=scale)
            c_new = ew.tile([128, 256], BF16, name="c", tag="c")
            tc_t = ew.tile([128, 256], BF16, name="tc_t", tag="tc_t")
            hT = ew.tile([128, KH, B], FP8, name="hT", tag="hT")
            if t == 0:
                nc.vector.tensor_tensor(c_new[:], sig_fi[:, 256:512], tg[:],
                                        op=ALU.mult)
                nc.scalar.activation(tc_t[:], c_new[:], AFT.Tanh)
                for hh in range(2):
                    nc.vector.tensor_tensor(
                        hT[:, 2 * hh:2 * hh + 2, :],
                        so[:, 128 * hh:128 * (hh + 1)]
                        .rearrange("p (k n) -> p k n", k=2),
                        tc_t[:, 128 * hh:128 * (hh + 1)]
                        .rearrange("p (k n) -> p k n", k=2),
                        op=ALU.mult)
            else:
                fc_ = ew.tile([128, 256], BF16, name="fc_", tag="fc_")
                u = ew.tile([128, 256], BF16, name="u", tag="u")
                for hh in range(2):
                    hs_ = slice(128 * hh, 128 * (hh + 1))
                    nc.vector.tensor_tensor(fc_[:, hs_], sig_fi[:, 0:256][:, hs_],
                                            state["c"][:][:, hs_], op=ALU.mult)
                    nc.vector.tensor_tensor(u[:, hs_], sig_fi[:, 256:512][:, hs_],
                                            tg[:, hs_], op=ALU.mult)
                    nc.vector.tensor_tensor(c_new[:, hs_], fc_[:, hs_],
                                            u[:, hs_], op=ALU.add)
                    nc.scalar.activation(tc_t[:, hs_], c_new[:, hs_], AFT.Tanh)
                    nc.vector.tensor_tensor(
                        hT[:, 2 * hh:2 * hh + 2, :],
                        so[:, hs_].rearrange("p (k n) -> p k n", k=2),
                        tc_t[:, hs_].rearrange("p (k n) -> p k n", k=2),
                        op=ALU.mult)
            nc.vector.tensor_tensor(
                h_seqT[:, :, t * B:(t + 1) * B],
                so[:].rearrange("p (k n) -> p k n", k=KH),
                tc_t[:].rearrange("p (k n) -> p k n", k=KH),
                op=ALU.mult)
            state["c"] = c_new
            state["hT"] = hT
            state["sig1"] = sig_fi

        fc_state = {}

        def fc_mms(m, chunks):
            pj = fc_state.get(m)
            if pj is None:
                pj = fc_state[m] = projp.tile([128, 1536], F32, name="pj",
                                              tag="pj")
            for ci in chunks:
                n0, nsz = FC_CHUNKS[ci]
                for k in range(KH):
                    nc.tensor.matmul(
                        pj[:, n0:n0 + nsz],
                        lhsT=h_seqT[:, k, m * 128:(m + 1) * 128],
                        rhs=fct[:, k, n0:n0 + nsz],
                        start=(k == 0),
                        stop=(k == KH - 1) and not with_fc_bias,
                        skip_group_check=True,
                    )
                if with_fc_bias:
                    nc.tensor.matmul(
                        pj[:, n0:n0 + nsz], lhsT=ones128[:],
                        rhs=fcb_sb[:, n0:n0 + nsz],
                        start=False, stop=True, skip_group_check=True,
                    )

        def fc_finish(m):
            pj = fc_state.pop(m)
            half = 640
            st_a = ost.tile([128, half], BF16, name="st_a", tag="st_a")
            st_b = ost.tile([128, VL - half], BF16, name="st_b", tag="st_b")
            nc.scalar.copy(st_a[:], pj[:, 0:half])
            nc.vector.tensor_copy(st_b[:], pj[:, half:VL])
            nc.sync.dma_start(out_d[m * 128:(m + 1) * 128, 0:half], st_a[:])
            nc.sync.dma_start(out_d[m * 128:(m + 1) * 128, half:VL], st_b[:])

        # ---- prologue ----
        gather_chunk(0)
        gather_chunk(1)
        for k in range(2 * KH):
            nc.sync.dma_start(w8[:, k, :], wco_d[k])
        gather_chunk(2)
        gather_chunk(3)
        for k in range(KH):
            nc.sync.dma_start(fct[:, k, :], fct_d[k])
        nc.scalar.copy(xsT0[:], ftst[:])  # f32 -> bf16 cast

        gates0 = gatesp.tile([128, 1024], F32, name="gates", tag="gates")
        for k in range(KH):
            for g in range(16):
                nc.tensor.matmul(
                    gates0[:, g * B:(g + 1) * B],
                    lhsT=wx0[:, k, g * 128:(g + 1) * 128],
                    rhs=xsT0[:, k * B:(k + 1) * B],
                    start=(k == 0 and g in (0, 8)),
                    stop=(k == KH - 1) and not with_gate_bias,
                    skip_group_check=True,
                )
        if with_gate_bias:
            for g in range(16):
                nc.tensor.matmul(
                    gates0[:, g * B:(g + 1) * B],
                    lhsT=bg1[:, g * 128:(g + 1) * 128], rhs=ones64[:],
                    start=False, stop=True, skip_group_check=True,
                )
        elementwise(0, gates0)

        gates_next = gatesp.tile([128, 1024], F32, name="gates", tag="gates")
        xmm(1, gates_next)

        # ---- scan ----
        if dbg:
            for k in range(KH):
                nc.sync.dma_start(dbg["h0"].ap()[:, k * B:(k + 1) * B],
                                  state["hT"][:, k, :])

        for t in range(1, T):
            gates = gates_next
            hmm(t, gates, state["hT"])
            if dbg and t == 1:
                gtmp = const.tile([128, 1024], BF16)
                nc.vector.tensor_copy(gtmp[:], gates[:])
                nc.sync.dma_start(dbg["g1"].ap(), gtmp[:])
                # recompute x-part and h-part separately into fresh psum
                for nm, base in (("gx1", 0), ("gh1", KH)):
                    gd = gatesp.tile([128, 1024], F32, name="gates", tag="gates")
                    for g2 in range(16):
                        for p in range(2):
                            rhs = (xsT[:, 2 * p:2 * p + 2, 1 * B:2 * B] if base == 0
                                   else state["hT"][:, 2 * p:2 * p + 2, :])
                            nc.tensor.matmul(
                                gd[:, g2 * B:(g2 + 1) * B],
                                lhsT=w8[:, base + 2 * p:base + 2 * p + 2,
                                        g2 * 128:(g2 + 1) * 128],
                                rhs=rhs, start=(p == 0), stop=(p == 1),
                                perf_mode=DR, skip_group_check=True)
                    gt2 = const.tile([128, 1024], BF16, name=f"gt_{nm}")
                    nc.vector.tensor_copy(gt2[:], gd[:])
                    nc.sync.dma_start(dbg[nm].ap(), gt2[:])
            if t % 2 == 0:
                fc_mms(t // 2 - 1, (0, 1))
            elif t >= 3:
                fc_mms((t - 3) // 2, (2,))
            if t < T - 1:
                gates_next = gatesp.tile([128, 1024], F32, name="gates",
                                         tag="gates")
                xmm(t + 1, gates_next)
            if t % 2 == 0 and 4 <= t // 2 + 3 < NCH:
                gather_chunk(t // 2 + 3)
            elementwise(t, gates)
            if t % 2 == 1 and t >= 3:
                fc_finish((t - 3) // 2)
            if dbg and t == 1:
                nc.sync.dma_start(dbg["sig1"].ap(), state["sig1"][:])

        if dbg:
            for k in range(KH):
                nc.sync.dma_start(dbg["xsT01"].ap()[:, k * 256:(k + 1) * 256],
                                  xsT[:, k, 0:256])
        # keep the PE p-state ramped through the final chain window so the
        # last projection prices at full clock
        warm = gatesp.tile([128, 1024], F32, name="gates", tag="gates")
        for w_i in range(14):
            nc.tensor.matmul(
                warm[:, 0:512], lhsT=wx0[:, w_i % KH, 0:128],
                rhs=fct[:, w_i % KH, 0:512],
                start=(w_i == 0), stop=(w_i == 13), skip_group_check=True,
            )
        fc_mms(NCH - 1, (0, 1, 2))
        fc_finish(NCH - 1)


_CACHE = {}


def _get_nc(with_gate_bias, with_fc_bias):
    key = (with_gate_bias, with_fc_bias)
    if key not in _CACHE:
        _CACHE[key] = build_nc(with_gate_bias, with_fc_bias)
    return _CACHE[key]


LAST_RESULTS = None

# gate reorder: pytorch rows (i,f,g,o) -> kernel order (f,i,g,o)
_PERM = np.concatenate([np.arange(H, 2 * H), np.arange(0, H),
                        np.arange(2 * H, 3 * H), np.arange(3 * H, 4 * H)])


def kernel(features, captions, embed_W, W_ih, W_hh, b_ih, b_hh, fc_W, fc_b,
           _trace=False):
    global LAST_RESULTS
    features = np.asarray(features, dtype=np.float32)
    captions = np.asarray(captions)
    embed_W = np.asarray(embed_W, dtype=np.float32)
    W_ih = np.asarray(W_ih, dtype=np.float32)
    W_hh = np.asarray(W_hh, dtype=np.float32)
    b_ih = np.asarray(b_ih, dtype=np.float32)
    b_hh = np.asarray(b_hh, dtype=np.float32)
    fc_W = np.asarray(fc_W, dtype=np.float32)
    fc_b = np.asarray(fc_b, dtype=np.float32)

    with_gate_bias = bool(np.any(b_ih) or np.any(b_hh))
    with_fc_bias = bool(np.any(fc_b))
    nc = _get_nc(with_gate_bias, with_fc_bias)

    FP8N = ml_dtypes.float8_e4m3fn
    BFN = ml_dtypes.bfloat16

    emb8 = (embed_W * SX).astype(BFN)

    # token-major indices, column m = tokens [128m, 128m+128); t=0 rows OOB
    tok = np.arange(NTOK)
    tt_, bb = tok // B, tok % B
    idx = np.where(tt_ == 0, OOB,
                   captions.astype(np.int64)[bb, tt_].astype(np.int64)
                   ).astype(np.int32)
    idx = np.ascontiguousarray(idx.reshape(NCH, 128).T)

    featT = np.ascontiguousarray(features.T.reshape(KH, 128, B))

    wxT = np.ascontiguousarray(W_ih.T[:, _PERM])          # [E, G4] reordered
    whT = np.ascontiguousarray(W_hh.T[:, _PERM])          # [H, G4]
    wco = np.concatenate([
        (wxT * SWX).astype(FP8N).reshape(KH, 128, G4),
        (whT * SWH).astype(FP8N).reshape(KH, 128, G4)], axis=0)
    wx0 = wxT.astype(BFN).reshape(KH, 128, G4)

    bg = (b_ih + b_hh)[_PERM].reshape(1, G4)

    fcT_full = np.ascontiguousarray(fc_W.T)               # [H, V]

    in_maps = []
    for c in range(NCORES):
        fct_c = np.ascontiguousarray(
            fcT_full[:, c * VL:(c + 1) * VL]).astype(BFN).reshape(KH, 128, VL)
        in_maps.append({
            "emb8": emb8,
            "idx": idx,
            "featT": featT,
            "wco": wco,
            "wx0": wx0,
            "fct": fct_c,
            "bg": bg,
            "fcb": fc_b[c * VL:(c + 1) * VL].reshape(1, VL),
        })

    try:
        res = run_bass_kernel_spmd(nc, in_maps, list(range(NCORES)),
                                   trace=_trace)
    except ModuleNotFoundError:
        res = run_bass_kernel_spmd(nc, in_maps, list(range(NCORES)))
    LAST_RESULTS = res

    outs = [
        np.asarray(res.results[c]["out"]).astype(np.float32)
        .reshape(T, B, VL).transpose(1, 0, 2)
        for c in range(NCORES)
    ]
    return np.ascontiguousarray(np.concatenate(outs, axis=2),
                                dtype=np.float32)


# revision 5
# speedup vs baseline: 1.0282x; 1.0230x over previous
"""Trainium2 Bass kernel for nn_DecoderRNN (LSTM decoder + vocab projection).

Strategy (8 NeuronCores, cost-model-driven rewrite):
  - Vocab-shard the output projection 8-way; replicate the LSTM recurrence.
  - Gate-major recurrence: gates.T [gate_dims, batch] so every matmul runs
    with a full M=128 stationary tile; moving dim N=64 (batch).  fp8-e4m3
    DoubleRow matmuls contract 2 K-planes per instruction at 0.5 cyc/row.
  - Scales: emb*8 (fp8), W_ih.T*16 (fp8), h unscaled (fp8), W_hh.T*128
    (fp8); PSUM holds 128*preact; activations apply scale=1/128.
    Timestep 0 (image features) runs in bf16 at scale 1.
  - Gate tiles ordered (f,i,g,o) so ACT runs 4 merged ops per step:
    SIG_fi [128,512], TANH_g [128,256], SIG_o [128,256], TANH_c [128,256].
  - Elementwise on DVE as bf16 tensor_tensor (2x mode); h written twice:
    fp8 (recurrence) + bf16 (h_seqT for the fc projection).
  - fc projection (bf16, N-chunks 512/512/226 in one 3-bank PSUM tile)
    paced into the scan (chunks 0-1 on even steps, chunk 2 + drain + DMA
    on odd steps); drained to two bf16 staging tiles split ACT/DVE.
  - Embeddings gathered on-device from a host-prescaled bf16 table,
    transposed via the PE, cast to fp8 in the drain, paced ahead of the
    scan.  One PSUM-bank start per matmul group (tiles 0 and 8) -- the
    hardware zeroes a whole 2KB bank per start_tensor_calc.
  - Post-TANH_g elementwise runs as two pipelined [128,128] half-chains
    so the h-part matmuls of the next step start on the first half while
    the second half finishes.  PE keep-warm matmuls before the tail
    projection hold the p-state at full clock.
"""

import numpy as np
import ml_dtypes

import concourse.bacc as bacc
import concourse.mybir as mybir
import concourse.tile as tile
from concourse.bass import IndirectOffsetOnAxis
from concourse.bass_utils import run_bass_kernel_spmd
from concourse.masks import make_identity

B, T, E, H, V = 64, 32, 512, 512, 10000
G4 = 4 * H            # 2048 gate dims (f,i,g,o ordered)
NTOK = B * T          # 2048 tokens
NCORES = 8
VL = V // NCORES      # 1250 vocab per core
KH = H // 128         # 4 K-chunks
NCH = NTOK // 128     # 16 gather chunks
OOB = 1 << 30         # gather sentinel for t=0 rows

SX = 8.0              # emb fp8 scale
SWX = 16.0            # W_ih fp8 scale
SWH = 128.0           # W_hh fp8 scale (h fp8 unscaled)
PS = SX * SWX         # PSUM preact scale for t>=1

F32 = mybir.dt.float32
BF16 = mybir.dt.bfloat16
FP8 = mybir.dt.float8e4
I32 = mybir.dt.int32
AFT = mybir.ActivationFunctionType
ALU = mybir.AluOpType
DR = mybir.MatmulPerfMode.DoubleRow

FC_CHUNKS = [(0, 512), (512, 512), (1024, VL - 1024)]


def build_nc(with_gate_bias: bool, with_fc_bias: bool):
    nc = bacc.Bacc("TRN2", target_bir_lowering=False, debug=False,
                   num_devices=NCORES)

    emb_d = nc.dram_tensor("emb8", [V, E], BF16, kind="ExternalInput")
    idx_d = nc.dram_tensor("idx", [128, NCH], I32, kind="ExternalInput")
    ft_d = nc.dram_tensor("featT", [KH, 128, B], F32, kind="ExternalInput")
    wco_d = nc.dram_tensor("wco", [2 * KH, 128, G4], FP8, kind="ExternalInput")
    wx0_d = nc.dram_tensor("wx0", [KH, 128, G4], BF16, kind="ExternalInput")
    fct_d = nc.dram_tensor("fct", [KH, 128, VL], BF16, kind="ExternalInput")
    bg_d = nc.dram_tensor("bg", [1, G4], F32, kind="ExternalInput")
    fcb_d = nc.dram_tensor("fcb", [1, VL], F32, kind="ExternalInput")
    out_d = nc.dram_tensor("out", [NTOK, VL], BF16, kind="ExternalOutput")
    import os
    dbg = {}
    if os.environ.get("K2_DEBUG"):
        dbg["xsT01"] = nc.dram_tensor("dbg_xsT01", [128, KH * 256], FP8, kind="ExternalOutput")
        dbg["h0"] = nc.dram_tensor("dbg_h0", [128, KH * B], FP8, kind="ExternalOutput")
        dbg["sig1"] = nc.dram_tensor("dbg_sig1", [128, 512], BF16, kind="ExternalOutput")
        dbg["gx1"] = nc.dram_tensor("dbg_gx1", [128, 1024], BF16, kind="ExternalOutput")
        dbg["gh1"] = nc.dram_tensor("dbg_gh1", [128, 1024], BF16, kind="ExternalOutput")
        dbg["g1"] = nc.dram_tensor("dbg_g1", [128, 1024], BF16, kind="ExternalOutput")

    with tile.TileContext(nc) as tc:
        build_body(nc, tc, emb_d, idx_d, ft_d, wco_d, wx0_d, fct_d, bg_d,
                   fcb_d, out_d, with_gate_bias, with_fc_bias, dbg)
    nc.compile()
    return nc


def build_body(nc, tc, emb_d, idx_d, ft_d, wco_d, wx0_d, fct_d, bg_d, fcb_d,
               out_d, with_gate_bias, with_fc_bias, dbg={}):
    from contextlib import ExitStack
    ctx = ExitStack()
    with ctx:
        const = ctx.enter_context(tc.tile_pool(name="const", bufs=1))
        gst = ctx.enter_context(tc.tile_pool(name="gst", bufs=3))
        ew = ctx.enter_context(tc.tile_pool(name="ew", bufs=2))
        ost = ctx.enter_context(tc.tile_pool(name="ost", bufs=2))
        gatesp = ctx.enter_context(tc.tile_pool(name="gatesp", bufs=2, space="PSUM"))
        projp = ctx.enter_context(tc.tile_pool(name="projp", bufs=1, space="PSUM"))
        tpsum = ctx.enter_context(tc.tile_pool(name="tpsum", bufs=1, space="PSUM"))

        # ---- constants / weights ----
        identb = const.tile([128, 128], BF16)
        make_identity(nc, identb[:])
        wrm = const.tile([128, 512], BF16)
        nc.gpsimd.memset(wrm[:], 0.25)

        idx_sb = const.tile([128, NCH], I32)
        nc.sync.dma_start(idx_sb[:], idx_d.ap())
        ftst = const.tile([128, KH * B], F32)
        for k in range(KH):
            nc.sync.dma_start(ftst[:, k * B:(k + 1) * B], ft_d[k])
        wx0 = const.tile([128, KH, G4], BF16)
        for k in range(KH):
            nc.sync.dma_start(wx0[:, k, 0:1024], wx0_d[k][:, 0:1024])
        for k in range(KH):
            nc.sync.dma_start(wx0[:, k, 1024:1536], wx0_d[k][:, 1024:1536])
        for k in range(KH):
            nc.sync.dma_start(wx0[:, k, 1536:G4], wx0_d[k][:, 1536:G4])
        w8 = const.tile([128, 2 * KH, G4], FP8)
        fct = const.tile([128, KH, VL], BF16)

        if with_gate_bias:
            bgrow = const.tile([1, G4], F32)
            nc.sync.dma_start(bgrow[:], bg_d.ap())
            bg128 = const.tile([1, G4], BF16)
            nc.vector.tensor_scalar_mul(bg128[:], bgrow[:], PS)
            bg1 = const.tile([1, G4], BF16)
            nc.vector.tensor_copy(bg1[:], bgrow[:])
            ones64 = const.tile([1, B], BF16)
            nc.gpsimd.memset(ones64[:], 1.0)
        if with_fc_bias:
            fcbrow = const.tile([1, VL], F32)
            nc.sync.dma_start(fcbrow[:], fcb_d.ap())
            fcb_sb = const.tile([1, VL], BF16)
            nc.vector.tensor_copy(fcb_sb[:], fcbrow[:])
            ones128 = const.tile([1, 128], BF16)
            nc.gpsimd.memset(ones128[:], 1.0)

        wps = gatesp.tile([128, 1024], F32, name="gates", tag="gates")
        for w_i in range(10):
            nc.tensor.matmul(wps[:, 0:512], lhsT=identb[:], rhs=wrm[:],
                             start=(w_i == 0), stop=(w_i == 9),
                             skip_group_check=True)

        xsT = const.tile([128, KH, NTOK], FP8)     # emb.T fp8, plane k
        xsT0 = const.tile([128, KH * B], BF16)     # features.T bf16
        h_seqT = const.tile([128, KH, NTOK], BF16)  # h.T bf16 for fc

        # ---- helpers ----
        drain_flip = [0]

        def gather_chunk(m):
            gs = gst.tile([128, E], BF16, name="gs", tag="gs")
            if m == 0:
                nc.gpsimd.memset(gs[:], 0.0)
            nc.gpsimd.indirect_dma_start(
                out=gs[:], out_offset=None, in_=emb_d.ap(),
                in_offset=IndirectOffsetOnAxis(ap=idx_sb[:, m:m + 1], axis=0),
                bounds_check=V - 1, oob_is_err=False,
            )
            tp = tpsum.tile([128, E], BF16, name="tp", tag="tp")
            for k in range(KH):
                nc.tensor.transpose(tp[:, k * 128:(k + 1) * 128],
                                    gs[:, k * 128:(k + 1) * 128], identb[:])
            dst = xsT[:, :, m * 128:(m + 1) * 128]
            src = tp[:].rearrange("p (k n) -> p k n", k=KH)
            if drain_flip[0] % 2 == 0:
                nc.scalar.copy(dst, src)
            else:
                nc.vector.tensor_copy(dst, src)
            drain_flip[0] += 1

        def xmm(t, gates):
            # x-part for step t (fp8 DR), opens each tile's PSUM group
            for g in range(16):
                for p in range(2):
                    nc.tensor.matmul(
                        gates[:, g * B:(g + 1) * B],
                        lhsT=w8[:, 2 * p:2 * p + 2, g * 128:(g + 1) * 128],
                        rhs=xsT[:, 2 * p:2 * p + 2, t * B:(t + 1) * B],
                        start=(p == 0 and g in (0, 8)), stop=False,
                        perf_mode=DR, skip_group_check=True,
                    )
            if with_gate_bias:
                for g in range(16):
                    nc.tensor.matmul(
                        gates[:, g * B:(g + 1) * B],
                        lhsT=bg128[:, g * 128:(g + 1) * 128],
                        rhs=ones64[:],
                        start=False, stop=False, skip_group_check=True,
                    )

        def hmm(t, gates, hT):
            # h-part for step t (fp8 DR), closes each tile's group.
            # q-major so q=0 runs while the second half of hT is produced;
            # f,i tiles (g 0..7) close first so SIG_fi starts earliest.
            for q in range(2):
                for g in range(16):
                    nc.tensor.matmul(
                        gates[:, g * B:(g + 1) * B],
                        lhsT=w8[:, KH + 2 * q:KH + 2 * q + 2,
                                g * 128:(g + 1) * 128],
                        rhs=hT[:, 2 * q:2 * q + 2, :],
                        start=False, stop=(q == 1),
                        perf_mode=DR, skip_group_check=True,
                    )

        state = {"c": None, "hT": None}

        def elementwise(t, gates):
            scale = 1.0 if t == 0 else 1.0 / PS
            sig_fi = ew.tile([128, 512], BF16, name="sig_fi", tag="sfi")
            nc.scalar.activation(sig_fi[:], gates[:, 0:512], AFT.Sigmoid,
                                 scale=scale)
            tg = ew.tile([128, 256], BF16, name="tg", tag="tg")
            nc.scalar.activation(tg[:], gates[:, 512:768], AFT.Tanh,
                                 scale=scale)
            so = ew.tile([128, 256], BF16, name="so", tag="so")
            nc.scalar.activation(so[:], gates[:, 768:1024], AFT.Sigmoid,
                                 scale=scale)
            c_new = ew.tile([128, 256], BF16, name="c", tag="c")
            tc_t = ew.tile([128, 256], BF16, name="tc_t", tag="tc_t")
            hT = ew.tile([128, KH, B], FP8, name="hT", tag="hT")
            if t == 0:
                nc.vector.tensor_tensor(c_new[:], sig_fi[:, 256:512], tg[:],
                                        op=ALU.mult)
                nc.scalar.activation(tc_t[:], c_new[:], AFT.Tanh)
                for hh in range(2):
                    nc.vector.tensor_tensor(
                        hT[:, 2 * hh:2 * hh + 2, :],
                        so[:, 128 * hh:128 * (hh + 1)]
                        .rearrange("p (k n) -> p k n", k=2),
                        tc_t[:, 128 * hh:128 * (hh + 1)]
                        .rearrange("p (k n) -> p k n", k=2),
                        op=ALU.mult)
            else:
                fc_ = ew.tile([128, 256], BF16, name="fc_", tag="fc_")
                u = ew.tile([128, 256], BF16, name="u", tag="u")
                for hh in range(2):
                    hs_ = slice(128 * hh, 128 * (hh + 1))
                    nc.vector.tensor_tensor(fc_[:, hs_], sig_fi[:, 0:256][:, hs_],
                                            state["c"][:][:, hs_], op=ALU.mult)
                    nc.vector.tensor_tensor(u[:, hs_], sig_fi[:, 256:512][:, hs_],
                                            tg[:, hs_], op=ALU.mult)
                    nc.vector.tensor_tensor(c_new[:, hs_], fc_[:, hs_],
                                            u[:, hs_], op=ALU.add)
                    nc.scalar.activation(tc_t[:, hs_], c_new[:, hs_], AFT.Tanh)
                    nc.vector.tensor_tensor(
                        hT[:, 2 * hh:2 * hh + 2, :],
                        so[:, hs_].rearrange("p (k n) -> p k n", k=2),
                        tc_t[:, hs_].rearrange("p (k n) -> p k n", k=2),
                        op=ALU.mult)
            nc.vector.tensor_tensor(
                h_seqT[:, :, t * B:(t + 1) * B],
                so[:].rearrange("p (k n) -> p k n", k=KH),
                tc_t[:].rearrange("p (k n) -> p k n", k=KH),
                op=ALU.mult)
            state["c"] = c_new
            state["hT"] = hT
            state["sig1"] = sig_fi

        fc_state = {}

        def fc_mms(m, chunks):
            pj = fc_state.get(m)
            if pj is None:
                pj = fc_state[m] = projp.tile([128, 1536], F32, name="pj",
                                              tag="pj")
            for ci in chunks:
                n0, nsz = FC_CHUNKS[ci]
                for k in range(KH):
                    nc.tensor.matmul(
                        pj[:, n0:n0 + nsz],
                        lhsT=h_seqT[:, k, m * 128:(m + 1) * 128],
                        rhs=fct[:, k, n0:n0 + nsz],
                        start=(k == 0),
                        stop=(k == KH - 1) and not with_fc_bias,
                        skip_group_check=True,
                    )
                if with_fc_bias:
                    nc.tensor.matmul(
                        pj[:, n0:n0 + nsz], lhsT=ones128[:],
                        rhs=fcb_sb[:, n0:n0 + nsz],
                        start=False, stop=True, skip_group_check=True,
                    )

        def fc_finish(m):
            pj = fc_state.pop(m)
            half = 640
            st_a = ost.tile([128, half], BF16, name="st_a", tag="st_a")
            st_b = ost.tile([128, VL - half], BF16, name="st_b", tag="st_b")
            nc.scalar.copy(st_a[:], pj[:, 0:half])
            nc.vector.tensor_copy(st_b[:], pj[:, half:VL])
            nc.sync.dma_start(out_d[m * 128:(m + 1) * 128, 0:half], st_a[:])
            nc.sync.dma_start(out_d[m * 128:(m + 1) * 128, half:VL], st_b[:])

        # ---- prologue ----
        gather_chunk(0)
        gather_chunk(1)
        for k in range(2 * KH):
            nc.sync.dma_start(w8[:, k, :], wco_d[k])
        gather_chunk(2)
        gather_chunk(3)
        for k in range(KH):
            nc.sync.dma_start(fct[:, k, :], fct_d[k])
        nc.scalar.copy(xsT0[:], ftst[:])  # f32 -> bf16 cast

        gates0 = gatesp.tile([128, 1024], F32, name="gates", tag="gates")
        for lo, hi in ((0, 8), (8, 12), (12, 16)):
            for k in range(KH):
                for g in range(lo, hi):
                    nc.tensor.matmul(
                        gates0[:, g * B:(g + 1) * B],
                        lhsT=wx0[:, k, g * 128:(g + 1) * 128],
                        rhs=xsT0[:, k * B:(k + 1) * B],
                        start=(k == 0 and g in (0, 8)),
                        stop=(k == KH - 1) and not with_gate_bias,
                        skip_group_check=True,
                    )
        if with_gate_bias:
            for g in range(16):
                nc.tensor.matmul(
                    gates0[:, g * B:(g + 1) * B],
                    lhsT=bg1[:, g * 128:(g + 1) * 128], rhs=ones64[:],
                    start=False, stop=True, skip_group_check=True,
                )
        elementwise(0, gates0)

        gates_next = gatesp.tile([128, 1024], F32, name="gates", tag="gates")
        xmm(1, gates_next)

        # ---- scan ----
        if dbg:
            for k in range(KH):
                nc.sync.dma_start(dbg["h0"].ap()[:, k * B:(k + 1) * B],
                                  state["hT"][:, k, :])

        for t in range(1, T):
            gates = gates_next
            hmm(t, gates, state["hT"])
            if dbg and t == 1:
                gtmp = const.tile([128, 1024], BF16)
                nc.vector.tensor_copy(gtmp[:], gates[:])
                nc.sync.dma_start(dbg["g1"].ap(), gtmp[:])
                # recompute x-part and h-part separately into fresh psum
                for nm, base in (("gx1", 0), ("gh1", KH)):
                    gd = gatesp.tile([128, 1024], F32, name="gates", tag="gates")
                    for g2 in range(16):
                        for p in range(2):
                            rhs = (xsT[:, 2 * p:2 * p + 2, 1 * B:2 * B] if base == 0
                                   else state["hT"][:, 2 * p:2 * p + 2, :])
                            nc.tensor.matmul(
                                gd[:, g2 * B:(g2 + 1) * B],
                                lhsT=w8[:, base + 2 * p:base + 2 * p + 2,
                                        g2 * 128:(g2 + 1) * 128],
                                rhs=rhs, start=(p == 0), stop=(p == 1),
                                perf_mode=DR, skip_group_check=True)
                    gt2 = const.tile([128, 1024], BF16, name=f"gt_{nm}")
                    nc.vector.tensor_copy(gt2[:], gd[:])
                    nc.sync.dma_start(dbg[nm].ap(), gt2[:])
            if t % 2 == 0:
                fc_mms(t // 2 - 1, (0, 1))
            elif t >= 3:
                fc_mms((t - 3) // 2, (2,))
            if t < T - 1:
                gates_next = gatesp.tile([128, 1024], F32, name="gates",
                                         tag="gates")
                xmm(t + 1, gates_next)
            if t % 2 == 0 and 4 <= t // 2 + 3 < NCH:
                gather_chunk(t // 2 + 3)
            elementwise(t, gates)
            if t % 2 == 1 and t >= 3:
                fc_finish((t - 3) // 2)
            if dbg and t == 1:
                nc.sync.dma_start(dbg["sig1"].ap(), state["sig1"][:])

        if dbg:
            for k in range(KH):
                nc.sync.dma_start(dbg["xsT01"].ap()[:, k * 256:(k + 1) * 256],
                                  xsT[:, k, 0:256])
        # keep the PE p-state ramped through the final chain window so the
        # last projection prices at full clock
        warm = gatesp.tile([128, 1024], F32, name="gates", tag="gates")
        for w_i in range(14):
            nc.tensor.matmul(
                warm[:, 0:512], lhsT=wx0[:, w_i % KH, 0:128],
                rhs=fct[:, w_i % KH, 0:512],
                start=(w_i == 0), stop=(w_i == 13), skip_group_check=True,
            )
        fc_mms(NCH - 1, (0, 1, 2))
        fc_finish(NCH - 1)


_CACHE = {}


def _get_nc(with_gate_bias, with_fc_bias):
    key = (with_gate_bias, with_fc_bias)
    if key not in _CACHE:
        _CACHE[key] = build_nc(with_gate_bias, with_fc_bias)
    return _CACHE[key]


LAST_RESULTS = None

# gate reorder: pytorch rows (i,f,g,o) -> kernel order (f,i,g,o)
_PERM = np.concatenate([np.arange(H, 2 * H), np.arange(0, H),
                        np.arange(2 * H, 3 * H), np.arange(3 * H, 4 * H)])


def kernel(features, captions, embed_W, W_ih, W_hh, b_ih, b_hh, fc_W, fc_b,
           _trace=False):
    global LAST_RESULTS
    features = np.asarray(features, dtype=np.float32)
    captions = np.asarray(captions)
    embed_W = np.asarray(embed_W, dtype=np.float32)
    W_ih = np.asarray(W_ih, dtype=np.float32)
    W_hh = np.asarray(W_hh, dtype=np.float32)
    b_ih = np.asarray(b_ih, dtype=np.float32)
    b_hh = np.asarray(b_hh, dtype=np.float32)
    fc_W = np.asarray(fc_W, dtype=np.float32)
    fc_b = np.asarray(fc_b, dtype=np.float32)

    with_gate_bias = bool(np.any(b_ih) or np.any(b_hh))
    with_fc_bias = bool(np.any(fc_b))
    nc = _get_nc(with_gate_bias, with_fc_bias)

    FP8N = ml_dtypes.float8_e4m3fn
    BFN = ml_dtypes.bfloat16

    emb8 = (embed_W * SX).astype(BFN)

    # token-major indices, column m = tokens [128m, 128m+128); t=0 rows OOB
    tok = np.arange(NTOK)
    tt_, bb = tok // B, tok % B
    idx = np.where(tt_ == 0, OOB,
                   captions.astype(np.int64)[bb, tt_].astype(np.int64)
                   ).astype(np.int32)
    idx = np.ascontiguousarray(idx.reshape(NCH, 128).T)

    featT = np.ascontiguousarray(features.T.reshape(KH, 128, B))

    wxT = np.ascontiguousarray(W_ih.T[:, _PERM])          # [E, G4] reordered
    whT = np.ascontiguousarray(W_hh.T[:, _PERM])          # [H, G4]
    wco = np.concatenate([
        (wxT * SWX).astype(FP8N).reshape(KH, 128, G4),
        (whT * SWH).astype(FP8N).reshape(KH, 128, G4)], axis=0)
    wx0 = wxT.astype(BFN).reshape(KH, 128, G4)

    bg = (b_ih + b_hh)[_PERM].reshape(1, G4)

    fcT_full = np.ascontiguousarray(fc_W.T)               # [H, V]

    in_maps = []
    for c in range(NCORES):
        fct_c = np.ascontiguousarray(
            fcT_full[:, c * VL:(c + 1) * VL]).astype(BFN).reshape(KH, 128, VL)
        in_maps.append({
            "emb8": emb8,
            "idx": idx,
            "featT": featT,
            "wco": wco,
            "wx0": wx0,
            "fct": fct_c,
            "bg": bg,
            "fcb": fc_b[c * VL:(c + 1) * VL].reshape(1, VL),
        })

    try:
        res = run_bass_kernel_spmd(nc, in_maps, list(range(NCORES)),
                                   trace=_trace)
    except ModuleNotFoundError:
        res = run_bass_kernel_spmd(nc, in_maps, list(range(NCORES)))
    LAST_RESULTS = res

    outs = [
        np.asarray(res.results[c]["out"]).astype(np.float32)
        .reshape(T, B, VL).transpose(1, 0, 2)
        for c in range(NCORES)
    ]
    return np.ascontiguousarray(np.concatenate(outs, axis=2),
                                dtype=np.float32)


# revision 6
# speedup vs baseline: 1.0346x; 1.0062x over previous
"""Trainium2 Bass kernel for nn_DecoderRNN (LSTM decoder + vocab projection).

Strategy (8 NeuronCores, cost-model-driven rewrite):
  - Vocab-shard the output projection 8-way; replicate the LSTM recurrence.
  - Gate-major recurrence: gates.T [gate_dims, batch] so every matmul runs
    with a full M=128 stationary tile; moving dim N=64 (batch).  fp8-e4m3
    DoubleRow matmuls contract 2 K-planes per instruction at 0.5 cyc/row.
  - Scales: emb*8 (fp8), W_ih.T*16 (fp8), h unscaled (fp8), W_hh.T*128
    (fp8); PSUM holds 128*preact; activations apply scale=1/128.
    Timestep 0 (image features) runs in bf16 at scale 1.
  - Gate tiles ordered (f,i,g,o) so ACT runs 4 merged ops per step:
    SIG_fi [128,512], TANH_g [128,256], SIG_o [128,256], TANH_c [128,256].
  - Elementwise on DVE as bf16 tensor_tensor (2x mode); h written twice:
    fp8 (recurrence) + bf16 (h_seqT for the fc projection).
  - fc projection (bf16, N-chunks 512/512/226 in one 3-bank PSUM tile)
    paced into the scan (chunks 0-1 on even steps, chunk 2 + drain + DMA
    on odd steps); drained to two bf16 staging tiles split ACT/DVE.
  - Embeddings gathered on-device from a host-prescaled bf16 table,
    transposed via the PE, cast to fp8 in the drain, paced ahead of the
    scan.  One PSUM-bank start per matmul group (tiles 0 and 8) -- the
    hardware zeroes a whole 2KB bank per start_tensor_calc.
  - Post-TANH_g elementwise runs as two pipelined [128,128] half-chains
    so the h-part matmuls of the next step start on the first half while
    the second half finishes.  PE keep-warm matmuls before the tail
    projection hold the p-state at full clock.
"""

import numpy as np
import ml_dtypes

import concourse.bacc as bacc
import concourse.mybir as mybir
import concourse.tile as tile
from concourse.bass import IndirectOffsetOnAxis
from concourse.bass_utils import run_bass_kernel_spmd
from concourse.masks import make_identity

B, T, E, H, V = 64, 32, 512, 512, 10000
G4 = 4 * H            # 2048 gate dims (f,i,g,o ordered)
NTOK = B * T          # 2048 tokens
NCORES = 8
VL = V // NCORES      # 1250 vocab per core
KH = H // 128         # 4 K-chunks
NCH = NTOK // 128     # 16 gather chunks
OOB = 1 << 30         # gather sentinel for t=0 rows

SX = 8.0              # emb fp8 scale
SWX = 16.0            # W_ih fp8 scale
SWH = 128.0           # W_hh fp8 scale (h fp8 unscaled)
PS = SX * SWX         # PSUM preact scale for t>=1

F32 = mybir.dt.float32
BF16 = mybir.dt.bfloat16
FP8 = mybir.dt.float8e4
I32 = mybir.dt.int32
AFT = mybir.ActivationFunctionType
ALU = mybir.AluOpType
DR = mybir.MatmulPerfMode.DoubleRow

FC_CHUNKS = [(0, 512), (512, 512), (1024, VL - 1024)]


def build_nc(with_gate_bias: bool, with_fc_bias: bool):
    nc = bacc.Bacc("TRN2", target_bir_lowering=False, debug=False,
                   num_devices=NCORES)

    emb_d = nc.dram_tensor("emb8", [V, E], BF16, kind="ExternalInput")
    idx_d = nc.dram_tensor("idx", [128, NCH], I32, kind="ExternalInput")
    ft_d = nc.dram_tensor("featT", [KH, 128, B], F32, kind="ExternalInput")
    wco_d = nc.dram_tensor("wco", [2 * KH, 128, G4], FP8, kind="ExternalInput")
    wx0_d = nc.dram_tensor("wx0", [KH, 128, G4], FP8, kind="ExternalInput")
    fct_d = nc.dram_tensor("fct", [KH, 128, VL], BF16, kind="ExternalInput")
    bg_d = nc.dram_tensor("bg", [1, G4], F32, kind="ExternalInput")
    fcb_d = nc.dram_tensor("fcb", [1, VL], F32, kind="ExternalInput")
    out_d = nc.dram_tensor("out", [NTOK, VL], BF16, kind="ExternalOutput")
    import os
    dbg = {}
    if os.environ.get("K2_DEBUG"):
        dbg["xsT01"] = nc.dram_tensor("dbg_xsT01", [128, KH * 256], FP8, kind="ExternalOutput")
        dbg["h0"] = nc.dram_tensor("dbg_h0", [128, KH * B], FP8, kind="ExternalOutput")
        dbg["sig1"] = nc.dram_tensor("dbg_sig1", [128, 512], BF16, kind="ExternalOutput")
        dbg["gx1"] = nc.dram_tensor("dbg_gx1", [128, 1024], BF16, kind="ExternalOutput")
        dbg["gh1"] = nc.dram_tensor("dbg_gh1", [128, 1024], BF16, kind="ExternalOutput")
        dbg["g1"] = nc.dram_tensor("dbg_g1", [128, 1024], BF16, kind="ExternalOutput")

    with tile.TileContext(nc) as tc:
        build_body(nc, tc, emb_d, idx_d, ft_d, wco_d, wx0_d, fct_d, bg_d,
                   fcb_d, out_d, with_gate_bias, with_fc_bias, dbg)
    nc.compile()
    return nc


def build_body(nc, tc, emb_d, idx_d, ft_d, wco_d, wx0_d, fct_d, bg_d, fcb_d,
               out_d, with_gate_bias, with_fc_bias, dbg={}):
    from contextlib import ExitStack
    ctx = ExitStack()
    with ctx:
        const = ctx.enter_context(tc.tile_pool(name="const", bufs=1))
        gst = ctx.enter_context(tc.tile_pool(name="gst", bufs=3))
        ew = ctx.enter_context(tc.tile_pool(name="ew", bufs=2))
        ost = ctx.enter_context(tc.tile_pool(name="ost", bufs=2))
        gatesp = ctx.enter_context(tc.tile_pool(name="gatesp", bufs=2, space="PSUM"))
        projp = ctx.enter_context(tc.tile_pool(name="projp", bufs=1, space="PSUM"))
        tpsum = ctx.enter_context(tc.tile_pool(name="tpsum", bufs=1, space="PSUM"))

        # ---- constants / weights ----
        identb = const.tile([128, 128], BF16)
        make_identity(nc, identb[:])
        wrm = const.tile([128, 512], BF16)
        nc.gpsimd.memset(wrm[:], 0.25)

        idx_sb = const.tile([128, NCH], I32)
        nc.sync.dma_start(idx_sb[:], idx_d.ap())
        ftst = const.tile([128, KH * B], F32)
        for k in range(KH):
            nc.sync.dma_start(ftst[:, k * B:(k + 1) * B], ft_d[k])
        w8 = const.tile([128, 2 * KH, G4], FP8)
        r8 = const.tile([128, KH, G4], FP8)
        for k in range(KH):
            nc.sync.dma_start(w8[:, k, :], wco_d[k])
        for k in range(KH):
            nc.sync.dma_start(r8[:, k, :], wx0_d[k])
        for k in range(KH, 2 * KH):
            nc.sync.dma_start(w8[:, k, :], wco_d[k])
        fct = const.tile([128, KH, VL], BF16)

        if with_gate_bias:
            bgrow = const.tile([1, G4], F32)
            nc.sync.dma_start(bgrow[:], bg_d.ap())
            bg128 = const.tile([1, G4], BF16)
            nc.vector.tensor_scalar_mul(bg128[:], bgrow[:], PS)
            bg16 = const.tile([1, G4], BF16)
            nc.vector.tensor_scalar_mul(bg16[:], bgrow[:], 16.0)
            ones64 = const.tile([1, B], BF16)
            nc.gpsimd.memset(ones64[:], 1.0)
        if with_fc_bias:
            fcbrow = const.tile([1, VL], F32)
            nc.sync.dma_start(fcbrow[:], fcb_d.ap())
            fcb_sb = const.tile([1, VL], BF16)
            nc.vector.tensor_copy(fcb_sb[:], fcbrow[:])
            ones128 = const.tile([1, 128], BF16)
            nc.gpsimd.memset(ones128[:], 1.0)

        wps = gatesp.tile([128, 1024], F32, name="gates", tag="gates")
        for w_i in range(10):
            nc.tensor.matmul(wps[:, 0:512], lhsT=identb[:], rhs=wrm[:],
                             start=(w_i == 0), stop=(w_i == 9),
                             skip_group_check=True)

        xsT = const.tile([128, KH, NTOK], FP8)     # emb.T fp8, plane k
        xsT0 = const.tile([128, KH * B], BF16)     # features.T bf16
        h_seqT = const.tile([128, KH, NTOK], BF16)  # h.T bf16 for fc

        # ---- helpers ----
        drain_flip = [0]

        def gather_chunk(m):
            gs = gst.tile([128, E], BF16, name="gs", tag="gs")
            if m == 0:
                nc.gpsimd.memset(gs[:], 0.0)
            nc.gpsimd.indirect_dma_start(
                out=gs[:], out_offset=None, in_=emb_d.ap(),
                in_offset=IndirectOffsetOnAxis(ap=idx_sb[:, m:m + 1], axis=0),
                bounds_check=V - 1, oob_is_err=False,
            )
            tp = tpsum.tile([128, E], BF16, name="tp", tag="tp")
            for k in range(KH):
                nc.tensor.transpose(tp[:, k * 128:(k + 1) * 128],
                                    gs[:, k * 128:(k + 1) * 128], identb[:])
            dst = xsT[:, :, m * 128:(m + 1) * 128]
            src = tp[:].rearrange("p (k n) -> p k n", k=KH)
            if drain_flip[0] % 2 == 0:
                nc.scalar.copy(dst, src)
            else:
                nc.vector.tensor_copy(dst, src)
            drain_flip[0] += 1

        def xmm(t, gates):
            # x-part for step t (fp8 DR), opens each tile's PSUM group
            for g in range(16):
                for p in range(2):
                    nc.tensor.matmul(
                        gates[:, g * B:(g + 1) * B],
                        lhsT=w8[:, 2 * p:2 * p + 2, g * 128:(g + 1) * 128],
                        rhs=xsT[:, 2 * p:2 * p + 2, t * B:(t + 1) * B],
                        start=(p == 0 and g in (0, 8)), stop=False,
                        perf_mode=DR, skip_group_check=True,
                    )
            if with_gate_bias:
                for g in range(16):
                    nc.tensor.matmul(
                        gates[:, g * B:(g + 1) * B],
                        lhsT=bg128[:, g * 128:(g + 1) * 128],
                        rhs=ones64[:],
                        start=False, stop=False, skip_group_check=True,
                    )

        def hmm(t, gates, hT):
            # h-part for step t (fp8 DR), closes each tile's group.
            # q-major so q=0 runs while the second half of hT is produced;
            # f,i tiles (g 0..7) close first so SIG_fi starts earliest.
            for q in range(2):
                for g in range(16):
                    nc.tensor.matmul(
                        gates[:, g * B:(g + 1) * B],
                        lhsT=w8[:, KH + 2 * q:KH + 2 * q + 2,
                                g * 128:(g + 1) * 128],
                        rhs=hT[:, 2 * q:2 * q + 2, :],
                        start=False, stop=(q == 1),
                        perf_mode=DR, skip_group_check=True,
                    )

        state = {"c": None, "hT": None}

        def elementwise(t, gates):
            scale = 1.0 / 16.0 if t == 0 else 1.0 / PS
            sig_fi = ew.tile([128, 512], BF16, name="sig_fi", tag="sfi")
            nc.scalar.activation(sig_fi[:], gates[:, 0:512], AFT.Sigmoid,
                                 scale=scale)
            tg = ew.tile([128, 256], BF16, name="tg", tag="tg")
            nc.scalar.activation(tg[:], gates[:, 512:768], AFT.Tanh,
                                 scale=scale)
            so = ew.tile([128, 256], BF16, name="so", tag="so")
            nc.scalar.activation(so[:], gates[:, 768:1024], AFT.Sigmoid,
                                 scale=scale)
            c_new = ew.tile([128, 256], BF16, name="c", tag="c")
            tc_t = ew.tile([128, 256], BF16, name="tc_t", tag="tc_t")
            hT = ew.tile([128, KH, B], FP8, name="hT", tag="hT")
            if t == 0:
                nc.vector.tensor_tensor(c_new[:], sig_fi[:, 256:512], tg[:],
                                        op=ALU.mult)
                nc.scalar.activation(tc_t[:], c_new[:], AFT.Tanh)
                for hh in range(2):
                    nc.vector.tensor_tensor(
                        hT[:, 2 * hh:2 * hh + 2, :],
                        so[:, 128 * hh:128 * (hh + 1)]
                        .rearrange("p (k n) -> p k n", k=2),
                        tc_t[:, 128 * hh:128 * (hh + 1)]
                        .rearrange("p (k n) -> p k n", k=2),
                        op=ALU.mult)
            else:
                fc_ = ew.tile([128, 256], BF16, name="fc_", tag="fc_")
                u = ew.tile([128, 256], BF16, name="u", tag="u")
                for hh in range(2):
                    hs_ = slice(128 * hh, 128 * (hh + 1))
                    nc.vector.tensor_tensor(fc_[:, hs_], sig_fi[:, 0:256][:, hs_],
                                            state["c"][:][:, hs_], op=ALU.mult)
                    nc.vector.tensor_tensor(u[:, hs_], sig_fi[:, 256:512][:, hs_],
                                            tg[:, hs_], op=ALU.mult)
                    nc.vector.tensor_tensor(c_new[:, hs_], fc_[:, hs_],
                                            u[:, hs_], op=ALU.add)
                    nc.scalar.activation(tc_t[:, hs_], c_new[:, hs_], AFT.Tanh)
                    nc.vector.tensor_tensor(
                        hT[:, 2 * hh:2 * hh + 2, :],
                        so[:, hs_].rearrange("p (k n) -> p k n", k=2),
                        tc_t[:, hs_].rearrange("p (k n) -> p k n", k=2),
                        op=ALU.mult)
            nc.vector.tensor_tensor(
                h_seqT[:, :, t * B:(t + 1) * B],
                so[:].rearrange("p (k n) -> p k n", k=KH),
                tc_t[:].rearrange("p (k n) -> p k n", k=KH),
                op=ALU.mult)
            state["c"] = c_new
            state["hT"] = hT
            state["sig1"] = sig_fi

        fc_state = {}

        def fc_mms(m, chunks):
            pj = fc_state.get(m)
            if pj is None:
                pj = fc_state[m] = projp.tile([128, 1536], F32, name="pj",
                                              tag="pj")
            for ci in chunks:
                n0, nsz = FC_CHUNKS[ci]
                for k in range(KH):
                    nc.tensor.matmul(
                        pj[:, n0:n0 + nsz],
                        lhsT=h_seqT[:, k, m * 128:(m + 1) * 128],
                        rhs=fct[:, k, n0:n0 + nsz],
                        start=(k == 0),
                        stop=(k == KH - 1) and not with_fc_bias,
                        skip_group_check=True,
                    )
                if with_fc_bias:
                    nc.tensor.matmul(
                        pj[:, n0:n0 + nsz], lhsT=ones128[:],
                        rhs=fcb_sb[:, n0:n0 + nsz],
                        start=False, stop=True, skip_group_check=True,
                    )

        def fc_finish(m):
            pj = fc_state.pop(m)
            half = 640
            st_a = ost.tile([128, half], BF16, name="st_a", tag="st_a")
            st_b = ost.tile([128, VL - half], BF16, name="st_b", tag="st_b")
            nc.scalar.copy(st_a[:], pj[:, 0:half])
            nc.vector.tensor_copy(st_b[:], pj[:, half:VL])
            nc.sync.dma_start(out_d[m * 128:(m + 1) * 128, 0:half], st_a[:])
            nc.sync.dma_start(out_d[m * 128:(m + 1) * 128, half:VL], st_b[:])

        # ---- prologue ----
        gather_chunk(0)
        gather_chunk(1)
        gather_chunk(2)
        gather_chunk(3)
        for k in range(KH):
            nc.sync.dma_start(fct[:, k, :], fct_d[k])
        nc.scalar.copy(xsT0[:], ftst[:])  # f32 -> bf16 cast

        gA = gatesp.tile([128, 1024], F32, name="gates", tag="gates")
        gB = gatesp.tile([128, 1024], F32, name="gates", tag="gates")
        for src, gt in ((w8, gA), (r8, gB)):
            for k in range(KH):
                for g in range(16):
                    nc.tensor.matmul(
                        gt[:, g * B:(g + 1) * B],
                        lhsT=src[:, k, g * 128:(g + 1) * 128],
                        rhs=xsT0[:, k * B:(k + 1) * B],
                        start=(k == 0 and g in (0, 8)),
                        stop=(k == KH - 1) and not (with_gate_bias
                                                    and gt is gA),
                        skip_group_check=True,
                    )
        if with_gate_bias:
            for g in range(16):
                nc.tensor.matmul(
                    gA[:, g * B:(g + 1) * B],
                    lhsT=bg16[:, g * 128:(g + 1) * 128], rhs=ones64[:],
                    start=False, stop=True, skip_group_check=True,
                )
        comb0 = const.tile([128, 1024], BF16)
        cA0 = const.tile([128, 1024], BF16)
        nc.scalar.copy(cA0[:], gA[:])
        for cc in range(2):
            nc.vector.scalar_tensor_tensor(
                comb0[:, 512 * cc:512 * (cc + 1)],
                gB[:, 512 * cc:512 * (cc + 1)], 1.0 / 32.0,
                cA0[:, 512 * cc:512 * (cc + 1)],
                op0=ALU.mult, op1=ALU.add)
        elementwise(0, comb0)

        gates_next = gatesp.tile([128, 1024], F32, name="gates", tag="gates")
        xmm(1, gates_next)

        # ---- scan ----
        if dbg:
            for k in range(KH):
                nc.sync.dma_start(dbg["h0"].ap()[:, k * B:(k + 1) * B],
                                  state["hT"][:, k, :])

        for t in range(1, T):
            gates = gates_next
            hmm(t, gates, state["hT"])
            if dbg and t == 1:
                gtmp = const.tile([128, 1024], BF16)
                nc.vector.tensor_copy(gtmp[:], gates[:])
                nc.sync.dma_start(dbg["g1"].ap(), gtmp[:])
                # recompute x-part and h-part separately into fresh psum
                for nm, base in (("gx1", 0), ("gh1", KH)):
                    gd = gatesp.tile([128, 1024], F32, name="gates", tag="gates")
                    for g2 in range(16):
                        for p in range(2):
                            rhs = (xsT[:, 2 * p:2 * p + 2, 1 * B:2 * B] if base == 0
                                   else state["hT"][:, 2 * p:2 * p + 2, :])
                            nc.tensor.matmul(
                                gd[:, g2 * B:(g2 + 1) * B],
                                lhsT=w8[:, base + 2 * p:base + 2 * p + 2,
                                        g2 * 128:(g2 + 1) * 128],
                                rhs=rhs, start=(p == 0), stop=(p == 1),
                                perf_mode=DR, skip_group_check=True)
                    gt2 = const.tile([128, 1024], BF16, name=f"gt_{nm}")
                    nc.vector.tensor_copy(gt2[:], gd[:])
                    nc.sync.dma_start(dbg[nm].ap(), gt2[:])
            if t % 2 == 0:
                fc_mms(t // 2 - 1, (0, 1))
            elif t >= 3:
                fc_mms((t - 3) // 2, (2,))
            if t < T - 1:
                gates_next = gatesp.tile([128, 1024], F32, name="gates",
                                         tag="gates")
                xmm(t + 1, gates_next)
            if t % 2 == 0 and 4 <= t // 2 + 3 < NCH:
                gather_chunk(t // 2 + 3)
            elementwise(t, gates)
            if t % 2 == 1 and t >= 3:
                fc_finish((t - 3) // 2)
            if dbg and t == 1:
                nc.sync.dma_start(dbg["sig1"].ap(), state["sig1"][:])

        if dbg:
            for k in range(KH):
                nc.sync.dma_start(dbg["xsT01"].ap()[:, k * 256:(k + 1) * 256],
                                  xsT[:, k, 0:256])
        # keep the PE p-state ramped through the final chain window so the
        # last projection prices at full clock
        warm = gatesp.tile([128, 1024], F32, name="gates", tag="gates")
        for w_i in range(14):
            nc.tensor.matmul(
                warm[:, 0:512], lhsT=identb[:],
                rhs=fct[:, w_i % KH, 0:512],
                start=(w_i == 0), stop=(w_i == 13), skip_group_check=True,
            )
        fc_mms(NCH - 1, (0, 1, 2))
        fc_finish(NCH - 1)


_CACHE = {}


def _get_nc(with_gate_bias, with_fc_bias):
    key = (with_gate_bias, with_fc_bias)
    if key not in _CACHE:
        _CACHE[key] = build_nc(with_gate_bias, with_fc_bias)
    return _CACHE[key]


LAST_RESULTS = None

# gate reorder: pytorch rows (i,f,g,o) -> kernel order (f,i,g,o)
_PERM = np.concatenate([np.arange(H, 2 * H), np.arange(0, H),
                        np.arange(2 * H, 3 * H), np.arange(3 * H, 4 * H)])


def kernel(features, captions, embed_W, W_ih, W_hh, b_ih, b_hh, fc_W, fc_b,
           _trace=False):
    global LAST_RESULTS
    features = np.asarray(features, dtype=np.float32)
    captions = np.asarray(captions)
    embed_W = np.asarray(embed_W, dtype=np.float32)
    W_ih = np.asarray(W_ih, dtype=np.float32)
    W_hh = np.asarray(W_hh, dtype=np.float32)
    b_ih = np.asarray(b_ih, dtype=np.float32)
    b_hh = np.asarray(b_hh, dtype=np.float32)
    fc_W = np.asarray(fc_W, dtype=np.float32)
    fc_b = np.asarray(fc_b, dtype=np.float32)

    with_gate_bias = bool(np.any(b_ih) or np.any(b_hh))
    with_fc_bias = bool(np.any(fc_b))
    nc = _get_nc(with_gate_bias, with_fc_bias)

    FP8N = ml_dtypes.float8_e4m3fn
    BFN = ml_dtypes.bfloat16

    emb8 = (embed_W * SX).astype(BFN)

    # token-major indices, column m = tokens [128m, 128m+128); t=0 rows OOB
    tok = np.arange(NTOK)
    tt_, bb = tok // B, tok % B
    idx = np.where(tt_ == 0, OOB,
                   captions.astype(np.int64)[bb, tt_].astype(np.int64)
                   ).astype(np.int32)
    idx = np.ascontiguousarray(idx.reshape(NCH, 128).T)

    featT = np.ascontiguousarray(features.T.reshape(KH, 128, B))

    wxT = np.ascontiguousarray(W_ih.T[:, _PERM])          # [E, G4] reordered
    whT = np.ascontiguousarray(W_hh.T[:, _PERM])          # [H, G4]
    wco = np.concatenate([
        (wxT * SWX).astype(FP8N).reshape(KH, 128, G4),
        (whT * SWH).astype(FP8N).reshape(KH, 128, G4)], axis=0)
    w8x_deq = (wxT * SWX).astype(FP8N).astype(np.float32) / SWX
    wx0 = ((wxT - w8x_deq) * 512.0).astype(FP8N).reshape(KH, 128, G4)

    bg = (b_ih + b_hh)[_PERM].reshape(1, G4)

    fcT_full = np.ascontiguousarray(fc_W.T)               # [H, V]

    in_maps = []
    for c in range(NCORES):
        fct_c = np.ascontiguousarray(
            fcT_full[:, c * VL:(c + 1) * VL]).astype(BFN).reshape(KH, 128, VL)
        in_maps.append({
            "emb8": emb8,
            "idx": idx,
            "featT": featT,
            "wco": wco,
            "wx0": wx0,
            "fct": fct_c,
            "bg": bg,
            "fcb": fc_b[c * VL:(c + 1) * VL].reshape(1, VL),
        })

    try:
        res = run_bass_kernel_spmd(nc, in_maps, list(range(NCORES)),
                                   trace=_trace)
    except ModuleNotFoundError:
        res = run_bass_kernel_spmd(nc, in_maps, list(range(NCORES)))
    LAST_RESULTS = res

    outs = [
        np.asarray(res.results[c]["out"]).astype(np.float32)
        .reshape(T, B, VL).transpose(1, 0, 2)
        for c in range(NCORES)
    ]
    return np.ascontiguousarray(np.concatenate(outs, axis=2),
                                dtype=np.float32)
